# revision 3
# baseline (speedup 1.0000x reference)
"""Trainium2 Bass kernel for nn_Block_40080634806275 (dense transformer block).

Strategy: pure data parallel over 8 NeuronCores; batch 1024 -> 128 rows/core.
Per core: LN1 -> QKV (f32r matmuls, activations-transposed stationary) ->
outer-product pseudo-attention computed via Taylor moments of exp (no 98x98
materialization) -> Wo -> LN2 -> W1+GELU -> W2.

LayerNorm affines are folded into the adjacent weight matrices on the host
(exact for the spec's ones/zeros fills); biases ride as an extra ones-row in
the stationary operand against an extra weight row.
"""

import math

import numpy as np

import concourse.bacc as bacc
import concourse.mybir as mybir
import concourse.tile as tile
from concourse.bass_utils import run_bass_kernel_spmd
from concourse.masks import make_identity

# ---- problem constants (hardcoded per spec) ----
B, D, H, HS = 1024, 1568, 16, 98
FF, DOUT = 6272, 784
NCORES = 8
BC = B // NCORES  # 128 batch rows per core
EPS = 1e-5
ATT_SCALE = float(D) ** -0.5
PT = 6            # Taylor order for exp (max |logit| ~0.26 -> err ~1.5e-8)
NT = 392          # output-column tile = 4 heads
NHG = 4           # head groups of 4 heads
NKD = 13          # K tiles over D: 12x128 + 1x(32+ones)
NKF = FF // 128   # 49 K tiles over FF

f32 = mybir.dt.float32
f32r = mybir.dt.float32r
AX = mybir.AxisListType
OP = mybir.AluOpType
AF = mybir.ActivationFunctionType

# K tiling of the D-contraction. (row0, n_weight_rows, n_feature_cols)
# Last tile carries the bias row: stationary [33, BC] = 32 features + ones row,
# weight rows 1536..1568 inclusive (32 features + bias).
KT_D = [(i * 128, 128, 128) for i in range(12)] + [(1536, 33, 32)]

_CACHE = {}


def _emit_ln(nc, lns, xt, ht, n, scratch):
    """LayerNorm (no affine) of xt (BC, n) -> ht, using scratch (BC, n)."""
    s1 = lns.tile([BC, 1], f32, tag="s1")
    nc.vector.tensor_reduce(out=s1[:], in_=xt, axis=AX.X, op=OP.add)
    nc.vector.tensor_tensor(out=scratch, in0=xt, in1=xt, op=OP.mult)
    s2 = lns.tile([BC, 1], f32, tag="s2")
    nc.vector.tensor_reduce(out=s2[:], in_=scratch, axis=AX.X, op=OP.add)
    mu = lns.tile([BC, 1], f32, tag="mu")
    nc.vector.tensor_scalar_mul(mu[:], s1[:], 1.0 / n)
    var = lns.tile([BC, 1], f32, tag="var")
    nc.vector.tensor_scalar_mul(var[:], s2[:], 1.0 / n)
    mu2 = lns.tile([BC, 1], f32, tag="mu2")
    nc.vector.tensor_tensor(out=mu2[:], in0=mu[:], in1=mu[:], op=OP.mult)
    nc.vector.tensor_tensor(out=var[:], in0=var[:], in1=mu2[:], op=OP.subtract)
    nc.vector.tensor_scalar_add(var[:], var[:], EPS)
    std = lns.tile([BC, 1], f32, tag="std")
    nc.scalar.activation(std[:], var[:], AF.Sqrt)
    rstd = lns.tile([BC, 1], f32, tag="rstd")
    nc.vector.reciprocal(rstd[:], std[:])
    nmu = lns.tile([BC, 1], f32, tag="nmu")
    nc.vector.scalar_tensor_tensor(
        out=nmu[:], in0=mu[:], scalar=-1.0, in1=rstd[:], op0=OP.mult, op1=OP.mult
    )
    nc.scalar.activation(ht, xt, AF.Identity, bias=nmu[:], scale=rstd[:])


def _build():
    nc = bacc.Bacc(None, target_bir_lowering=False)

    x_d = nc.dram_tensor("x", [BC, D], f32, kind="ExternalInput")
    wqkv_d = nc.dram_tensor("wqkv", [D + 1, 3 * D], f32r, kind="ExternalInput")
    wo_d = nc.dram_tensor("wo", [D + 1, D], f32r, kind="ExternalInput")
    w1_d = nc.dram_tensor("w1", [D + 1, FF], f32r, kind="ExternalInput")
    w2_d = nc.dram_tensor("w2", [FF + 1, DOUT], f32r, kind="ExternalInput")
    y_d = nc.dram_tensor("y", [BC, DOUT], f32, kind="ExternalOutput")

    with tile.TileContext(nc) as tc:
        with (
            tc.tile_pool(name="const", bufs=1) as constp,
            tc.tile_pool(name="acts", bufs=1) as acts,
            tc.tile_pool(name="lns", bufs=2) as lns,
            tc.tile_pool(name="att", bufs=2) as att,
            tc.tile_pool(name="mom", bufs=2) as mom,
            tc.tile_pool(name="statT", bufs=13) as statT,
            tc.tile_pool(name="aTp", bufs=8) as aTp,
            tc.tile_pool(name="gTp", bufs=4) as gTp,
            tc.tile_pool(name="wt", bufs=16) as wtp,
            tc.tile_pool(name="psQ", bufs=2, space="PSUM") as psQ,
            tc.tile_pool(name="psT", bufs=2, space="PSUM") as psT,
            tc.tile_pool(name="psB", bufs=4, space="PSUM") as psB,
        ):
            ident = constp.tile([128, 128], f32)
            make_identity(nc, ident[:])
            ones_r = constp.tile([1, BC], f32r)
            nc.vector.tensor_copy(ones_r[:], nc.const_aps.tensor(1.0, (1, BC)))

            # ---- load x, LN1 ----
            xs = acts.tile([BC, D], f32, tag="xs")
            nc.sync.dma_start(xs[:], x_d[:])
            scratch = acts.tile([BC, D], f32, tag="scratch")
            h = acts.tile([BC, D], f32, tag="h")
            _emit_ln(nc, lns, xs[:], h[:], D, scratch[:])

            def stat_transposes(src, tag):
                """Transpose (BC, D) src into 13 stationary K tiles (f32r)."""
                tiles = []
                for r0, nrw, nf in KT_D:
                    st = statT.tile([nrw, BC], f32r, tag=tag)
                    pst = psT.tile([nf, BC], f32, tag="tr")
                    nc.tensor.transpose(pst[:], src[:, r0 : r0 + nf], ident[:])
                    nc.vector.tensor_copy(st[0:nf, :], pst[:])
                    if nrw == nf + 1:  # ones row for bias
                        nc.vector.tensor_copy(
                            st[nf : nf + 1, :], nc.const_aps.tensor(1.0, (1, BC))
                        )
                    tiles.append(st)
                return tiles

            hT = stat_transposes(h, "stat")

            # ---- QKV matmuls (k, v first, then q) ----
            tq = acts.tile([BC, D], f32, tag="tq")
            ksb = acts.tile([BC, D], f32, tag="ksb")
            vsb = acts.tile([BC, D], f32, tag="vsb")

            def mm_group(psum_ap, stats, wdram, col0, ncols, kt, start, stop):
                for i, (r0, nrw, _nf) in enumerate(kt):
                    wt = wtp.tile([nrw, ncols], f32r, tag="w")
                    nc.sync.dma_start(wt[:], wdram[r0 : r0 + nrw, col0 : col0 + ncols])
                    nc.tensor.matmul(
                        psum_ap,
                        stats[i][:],
                        wt[:],
                        start=start and i == 0,
                        stop=stop and i == len(kt) - 1,
                    )

            for hg in range(NHG):
                c0 = hg * NT
                for base, dst, scl in ((D, ksb, None), (2 * D, vsb, None), (0, tq, ATT_SCALE)):
                    ps = psQ.tile([BC, NT], f32, tag="mm")
                    mm_group(ps[:], hT, wqkv_d, base + c0, NT, KT_D, True, True)
                    if scl is None:
                        nc.scalar.copy(dst[:, c0 : c0 + NT], ps[:])
                    else:
                        nc.scalar.mul(dst[:, c0 : c0 + NT], ps[:], scl)

            # ---- attention via exp-Taylor moments, head-group pipelined ----
            attn = acts.tile([BC, D], f32, tag="attn")
            ps_wo = [psB.tile([BC, NT], f32, tag="acc", name=f"ps_wo{n}") for n in range(4)]

            for hg in range(NHG):
                c0 = hg * NT
                k2 = ksb[:, c0 : c0 + NT]
                v2 = vsb[:, c0 : c0 + NT]
                t2 = tq[:, c0 : c0 + NT]
                k3 = k2.rearrange("p (h j) -> p h j", j=HS)
                v3 = v2.rearrange("p (h j) -> p h j", j=HS)

                M = [mom.tile([BC, 4], f32, tag=f"M{p}", name=f"M{p}") for p in range(PT + 1)]
                N = [None] + [mom.tile([BC, 4], f32, tag=f"N{p}", name=f"N{p}") for p in range(1, PT + 1)]

                nc.vector.tensor_reduce(out=M[0][:], in_=v3, axis=AX.X, op=OP.add)
                nc.vector.tensor_reduce(out=N[1][:], in_=k3, axis=AX.X, op=OP.add)
                kv = att.tile([BC, NT], f32, tag="kv")
                nc.vector.tensor_tensor(out=kv[:], in0=k2, in1=v2, op=OP.mult)
                kv3 = kv[:].rearrange("p (h j) -> p h j", j=HS)
                nc.vector.tensor_reduce(out=M[1][:], in_=kv3, axis=AX.X, op=OP.add)
                kp = att.tile([BC, NT], f32, tag="kp")
                kp3 = kp[:].rearrange("p (h j) -> p h j", j=HS)
                nc.vector.tensor_tensor(out=kp[:], in0=k2, in1=k2, op=OP.mult)
                for p in range(2, PT + 1):
                    if p > 2:
                        nc.vector.tensor_tensor(out=kp[:], in0=kp[:], in1=k2, op=OP.mult)
                    nc.vector.tensor_reduce(out=N[p][:], in_=kp3, axis=AX.X, op=OP.add)
                    nc.vector.tensor_tensor(out=kv[:], in0=kp[:], in1=v2, op=OP.mult)
                    nc.vector.tensor_reduce(out=M[p][:], in_=kv3, axis=AX.X, op=OP.add)
                for p in range(2, PT + 1):
                    c = 1.0 / math.factorial(p)
                    nc.vector.tensor_scalar_mul(M[p][:], M[p][:], c)
                    nc.vector.tensor_scalar_mul(N[p][:], N[p][:], c)

                def bc3(m):
                    return m[:].unsqueeze(2).to_broadcast((BC, 4, HS))

                na = att.tile([BC, NT], f32, tag="na")
                na3 = na[:].rearrange("p (h j) -> p h j", j=HS)
                nc.vector.tensor_copy(na3, bc3(M[PT]))
                for p in range(PT - 1, -1, -1):
                    nc.vector.tensor_tensor(out=na[:], in0=na[:], in1=t2, op=OP.mult)
                    nc.vector.tensor_tensor(out=na3, in0=na3, in1=bc3(M[p]), op=OP.add)
                da = att.tile([BC, NT], f32, tag="da")
                da3 = da[:].rearrange("p (h j) -> p h j", j=HS)
                nc.vector.tensor_copy(da3, bc3(N[PT]))
                for p in range(PT - 1, 0, -1):
                    nc.vector.tensor_tensor(out=da[:], in0=da[:], in1=t2, op=OP.mult)
                    nc.vector.tensor_tensor(out=da3, in0=da3, in1=bc3(N[p]), op=OP.add)
                nc.vector.tensor_tensor(out=da[:], in0=da[:], in1=t2, op=OP.mult)
                nc.vector.tensor_scalar_add(da[:], da[:], float(HS))
                rec = att.tile([BC, NT], f32, tag="rec")
                nc.vector.reciprocal(rec[:], da[:])
                nc.vector.tensor_tensor(
                    out=attn[:, c0 : c0 + NT], in0=na[:], in1=rec[:], op=OP.mult
                )

                # transposes of this head group + Wo partial accumulation
                aT = []
                for j in range(4):
                    head = 4 * hg + j
                    st = aTp.tile([HS, BC], f32r, tag="aT")
                    pst = psT.tile([HS, BC], f32, tag="tr")
                    nc.tensor.transpose(
                        pst[:], attn[:, head * HS : (head + 1) * HS], ident[:]
                    )
                    nc.vector.tensor_copy(st[:], pst[:])
                    aT.append(st)
                for n in range(4):
                    for j in range(4):
                        head = 4 * hg + j
                        wt = wtp.tile([HS, NT], f32r, tag="w")
                        nc.sync.dma_start(
                            wt[:], wo_d[head * HS : head * HS + HS, n * NT : (n + 1) * NT]
                        )
                        nc.tensor.matmul(
                            ps_wo[n][:],
                            aT[j][:],
                            wt[:],
                            start=(hg == 0 and j == 0),
                            stop=False,
                        )

            # ---- Wo bias row, out copies, LN2 ----
            o = acts.tile([BC, D], f32, tag="o")
            for n in range(4):
                wt = wtp.tile([1, NT], f32r, tag="w", name=f"wob{n}")
                nc.sync.dma_start(wt[:], wo_d[D : D + 1, n * NT : (n + 1) * NT])
                nc.tensor.matmul(ps_wo[n][:], ones_r[:], wt[:], start=False, stop=True)
            for n in range(4):
                nc.scalar.copy(o[:, n * NT : (n + 1) * NT], ps_wo[n][:])
            h2 = acts.tile([BC, D], f32, tag="h2")
            _emit_ln(nc, lns, o[:], h2[:], D, scratch[:])
            h2T = stat_transposes(h2, "stat")

            # ---- W1 + GELU ----
            g = acts.tile([BC, FF], f32, tag="g")
            for n in range(FF // NT):
                ps = psQ.tile([BC, NT], f32, tag="mm")
                mm_group(ps[:], h2T, w1_d, n * NT, NT, KT_D, True, True)
                nc.scalar.activation(g[:, n * NT : (n + 1) * NT], ps[:], AF.Gelu)

            # ---- W2 (stream transposes of g) ----
            ps_w2 = [psB.tile([BC, NT], f32, tag="acc", name=f"ps_w2{n}") for n in range(2)]
            for kk in range(NKF):
                gT = gTp.tile([128, BC], f32r, tag="gT")
                pst = psT.tile([128, BC], f32, tag="tr")
                nc.tensor.transpose(pst[:], g[:, kk * 128 : (kk + 1) * 128], ident[:])
                nc.vector.tensor_copy(gT[:], pst[:])
                for n in range(2):
                    wt = wtp.tile([128, NT], f32r, tag="w")
                    nc.sync.dma_start(
                        wt[:], w2_d[kk * 128 : (kk + 1) * 128, n * NT : (n + 1) * NT]
                    )
                    nc.tensor.matmul(
                        ps_w2[n][:], gT[:], wt[:], start=(kk == 0), stop=False
                    )
            for n in range(2):
                wt = wtp.tile([1, NT], f32r, tag="w")
                nc.sync.dma_start(wt[:], w2_d[FF : FF + 1, n * NT : (n + 1) * NT])
                nc.tensor.matmul(ps_w2[n][:], ones_r[:], wt[:], start=False, stop=True)

            ff = acts.tile([BC, DOUT], f32, tag="ff")
            for n in range(2):
                nc.scalar.copy(ff[:, n * NT : (n + 1) * NT], ps_w2[n][:])
            nc.sync.dma_start(y_d[:], ff[:])

    nc.compile()
    return nc


def _prep_weights(Wq, Wk, Wv, Wo, bo, g1, b1, g2, b2, W1, b1f, W2, b2f):
    """Fold LN affines into adjacent weights; append bias rows. float64 exact."""
    f8 = np.float64
    wq = np.asarray(Wq, f8).transpose(1, 0, 2).reshape(D, D)
    wk = np.asarray(Wk, f8).transpose(1, 0, 2).reshape(D, D)
    wv = np.asarray(Wv, f8).transpose(1, 0, 2).reshape(D, D)
    wqkv = np.concatenate([wq, wk, wv], axis=1)  # (D, 3D)
    g1 = np.asarray(g1, f8)
    b1 = np.asarray(b1, f8)
    wqkv_aug = np.concatenate([g1[:, None] * wqkv, (b1 @ wqkv)[None, :]], axis=0)
    wo_aug = np.concatenate(
        [np.asarray(Wo, f8), np.asarray(bo, f8)[None, :]], axis=0
    )
    g2 = np.asarray(g2, f8)
    b2 = np.asarray(b2, f8)
    W1 = np.asarray(W1, f8)
    w1_aug = np.concatenate(
        [g2[:, None] * W1, (b2 @ W1 + np.asarray(b1f, f8))[None, :]], axis=0
    )
    w2_aug = np.concatenate(
        [np.asarray(W2, f8), np.asarray(b2f, f8)[None, :]], axis=0
    )
    return (
        wqkv_aug.astype(np.float32),
        wo_aug.astype(np.float32),
        w1_aug.astype(np.float32),
        w2_aug.astype(np.float32),
    )


def kernel(**inputs) -> np.ndarray:
    if "nc" not in _CACHE:
        _CACHE["nc"] = _build()
    nc = _CACHE["nc"]

    x = np.ascontiguousarray(np.asarray(inputs["x"], np.float32))
    wqkv_aug, wo_aug, w1_aug, w2_aug = _prep_weights(
        inputs["Wq"], inputs["Wk"], inputs["Wv"], inputs["Wo"], inputs["bo"],
        inputs["g1"], inputs["b1"], inputs["g2"], inputs["b2"],
        inputs["W1"], inputs["b1f"], inputs["W2"], inputs["b2f"],
    )
    in_maps = [
        {
            "x": x[c * BC : (c + 1) * BC],
            "wqkv": wqkv_aug,
            "wo": wo_aug,
            "w1": w1_aug,
            "w2": w2_aug,
        }
        for c in range(NCORES)
    ]
    res = run_bass_kernel_spmd(nc, in_maps, core_ids=list(range(NCORES)), trace=False)
    return np.concatenate([res.results[c]["y"] for c in range(NCORES)], axis=0)


# revision 4
# speedup vs baseline: 1.1936x; 1.1936x over previous
"""Trainium2 Bass kernel for nn_Block_40080634806275 (dense transformer block).

Strategy: pure data parallel over 8 NeuronCores; batch 1024 -> 128 rows/core.
Per core: LN1 -> QKV (f32r matmuls, activations-transposed stationary) ->
outer-product pseudo-attention computed via Taylor moments of exp (no 98x98
materialization) -> Wo -> LN2 -> W1+GELU -> W2.

LayerNorm affines are folded into the adjacent weight matrices on the host
(exact for the spec's ones/zeros fills); biases ride as an extra ones-row
matmul against an extra weight row. Weight DMAs use wide tiles (3136/6272-byte
partition lines) to reach the ~360 GB/s per-core HBM ceiling.
"""

import math

import numpy as np

import concourse.bacc as bacc
import concourse.mybir as mybir
import concourse.tile as tile
from concourse.bass_utils import run_bass_kernel_spmd
from concourse.masks import make_identity

# ---- problem constants (hardcoded per spec) ----
B, D, H, HS = 1024, 1568, 16, 98
FF, DOUT = 6272, 784
NCORES = 8
BC = B // NCORES  # 128 batch rows per core
EPS = 1e-5
ATT_SCALE = float(D) ** -0.5
PT = 6            # Taylor order for exp (max |logit| ~0.26 -> err ~1.5e-8)
NT = 392          # output-column tile = 4 heads
NHG = 4           # head groups of 4 heads
NKF = FF // 128   # 49 K tiles over FF

f32 = mybir.dt.float32
f32r = mybir.dt.float32r
AX = mybir.AxisListType
OP = mybir.AluOpType
AF = mybir.ActivationFunctionType

# K tiling of the D-contraction. (row0, n_weight_rows, n_feature_cols)
# Last tile carries the bias row: stationary [33, BC] = 32 features + ones row,
# weight rows 1536..1568 inclusive (32 features + bias).
KT_D = [(i * 128, 128, 128) for i in range(12)] + [(1536, 33, 32)]

_CACHE = {}


def _emit_ln(nc, lns, xt, ht, n, scratch):
    """LayerNorm (no affine) of xt (BC, n) -> ht, using scratch (BC, n)."""
    s1 = lns.tile([BC, 1], f32, tag="s1")
    nc.vector.tensor_reduce(out=s1[:], in_=xt, axis=AX.X, op=OP.add)
    nc.vector.tensor_tensor(out=scratch, in0=xt, in1=xt, op=OP.mult)
    s2 = lns.tile([BC, 1], f32, tag="s2")
    nc.vector.tensor_reduce(out=s2[:], in_=scratch, axis=AX.X, op=OP.add)
    mu = lns.tile([BC, 1], f32, tag="mu")
    nc.vector.tensor_scalar_mul(mu[:], s1[:], 1.0 / n)
    var = lns.tile([BC, 1], f32, tag="var")
    nc.vector.tensor_scalar_mul(var[:], s2[:], 1.0 / n)
    mu2 = lns.tile([BC, 1], f32, tag="mu2")
    nc.vector.tensor_tensor(out=mu2[:], in0=mu[:], in1=mu[:], op=OP.mult)
    nc.vector.tensor_tensor(out=var[:], in0=var[:], in1=mu2[:], op=OP.subtract)
    nc.vector.tensor_scalar_add(var[:], var[:], EPS)
    std = lns.tile([BC, 1], f32, tag="std")
    nc.scalar.activation(std[:], var[:], AF.Sqrt)
    rstd = lns.tile([BC, 1], f32, tag="rstd")
    nc.vector.reciprocal(rstd[:], std[:])
    nmu = lns.tile([BC, 1], f32, tag="nmu")
    nc.vector.scalar_tensor_tensor(
        out=nmu[:], in0=mu[:], scalar=-1.0, in1=rstd[:], op0=OP.mult, op1=OP.mult
    )
    nc.scalar.activation(ht, xt, AF.Identity, bias=nmu[:], scale=rstd[:])


def _build():
    nc = bacc.Bacc(None, target_bir_lowering=False)

    x_d = nc.dram_tensor("x", [BC, D], f32, kind="ExternalInput")
    wqkv_d = nc.dram_tensor("wqkv", [D + 1, 3 * D], f32r, kind="ExternalInput")
    wo_d = nc.dram_tensor("wo", [D + 1, D], f32r, kind="ExternalInput")
    w1_d = nc.dram_tensor("w1", [D + 1, FF], f32r, kind="ExternalInput")
    w2_d = nc.dram_tensor("w2", [FF + 1, DOUT], f32r, kind="ExternalInput")
    y_d = nc.dram_tensor("y", [BC, DOUT], f32, kind="ExternalOutput")

    with tile.TileContext(nc) as tc:
        with (
            tc.tile_pool(name="const", bufs=1) as constp,
            tc.tile_pool(name="acts", bufs=1) as acts,
            tc.tile_pool(name="lns", bufs=2) as lns,
            tc.tile_pool(name="att", bufs=2) as att,
            tc.tile_pool(name="mom", bufs=2) as mom,
            tc.tile_pool(name="statT", bufs=13) as statT,
            tc.tile_pool(name="aTp", bufs=8) as aTp,
            tc.tile_pool(name="gTp", bufs=4) as gTp,
            tc.tile_pool(name="wt", bufs=10) as wtp,     # [*, 784] tiles
            tc.tile_pool(name="wt1", bufs=6) as wtp1,    # [*, 1568] tiles (W1)
            tc.tile_pool(name="psQ", bufs=2, space="PSUM") as psQ,
            tc.tile_pool(name="psT", bufs=2, space="PSUM") as psT,
            tc.tile_pool(name="psB", bufs=4, space="PSUM") as psB,
        ):
            ident = constp.tile([128, 128], f32)
            make_identity(nc, ident[:])
            ones_r = constp.tile([1, BC], f32r)
            nc.vector.tensor_copy(ones_r[:], nc.const_aps.tensor(1.0, (1, BC)))

            # ---- load x, LN1 ----
            xs = acts.tile([BC, D], f32, tag="xs")
            nc.sync.dma_start(xs[:], x_d[:])
            scratch = acts.tile([BC, D], f32, tag="scratch")
            h = acts.tile([BC, D], f32, tag="h")
            _emit_ln(nc, lns, xs[:], h[:], D, scratch[:])

            def stat_transposes(src, tag):
                """Transpose (BC, D) src into 13 stationary K tiles (f32r)."""
                tiles = []
                for r0, nrw, nf in KT_D:
                    st = statT.tile([nrw, BC], f32r, tag=tag, name="st")
                    pst = psT.tile([nf, BC], f32, tag="tr", name="pst")
                    nc.tensor.transpose(pst[:], src[:, r0 : r0 + nf], ident[:])
                    nc.vector.tensor_copy(st[0:nf, :], pst[:])
                    if nrw == nf + 1:  # ones row for bias
                        nc.vector.tensor_copy(
                            st[nf : nf + 1, :], nc.const_aps.tensor(1.0, (1, BC))
                        )
                    tiles.append(st)
                return tiles

            hT = stat_transposes(h, "stat")

            # ---- QKV matmuls: per hg-pair, per tensor, 784-wide weight DMAs ----
            tq = acts.tile([BC, D], f32, tag="tq")
            ksb = acts.tile([BC, D], f32, tag="ksb")
            vsb = acts.tile([BC, D], f32, tag="vsb")

            for pair in range(2):
                p0 = pair * 2 * NT  # 0 or 784
                for base, dst, scl in (
                    (D, ksb, None), (2 * D, vsb, None), (0, tq, ATT_SCALE)
                ):
                    psE = psQ.tile([BC, NT], f32, tag="mm", name="psE")
                    psO = psQ.tile([BC, NT], f32, tag="mm", name="psO")
                    for i, (r0, nrw, _nf) in enumerate(KT_D):
                        wt = wtp.tile([nrw, 2 * NT], f32r, tag="w", name="wqkv_t")
                        nc.sync.dma_start(
                            wt[:], wqkv_d[r0 : r0 + nrw, base + p0 : base + p0 + 2 * NT]
                        )
                        nc.tensor.matmul(
                            psE[:], hT[i][:], wt[:, 0:NT],
                            start=i == 0, stop=i == len(KT_D) - 1,
                        )
                        nc.tensor.matmul(
                            psO[:], hT[i][:], wt[:, NT : 2 * NT],
                            start=i == 0, stop=i == len(KT_D) - 1,
                        )
                    for ps, off in ((psE, 0), (psO, NT)):
                        if scl is None:
                            nc.scalar.copy(dst[:, p0 + off : p0 + off + NT], ps[:])
                        else:
                            nc.scalar.mul(dst[:, p0 + off : p0 + off + NT], ps[:], scl)

            # ---- attention via exp-Taylor moments, head-group pipelined ----
            attn = acts.tile([BC, D], f32, tag="attn")
            ps_wo = [psB.tile([BC, NT], f32, tag="acc", name=f"ps_wo{n}") for n in range(4)]

            for hg in range(NHG):
                c0 = hg * NT
                k2 = ksb[:, c0 : c0 + NT]
                v2 = vsb[:, c0 : c0 + NT]
                t2 = tq[:, c0 : c0 + NT]
                k3 = k2.rearrange("p (h j) -> p h j", j=HS)
                v3 = v2.rearrange("p (h j) -> p h j", j=HS)

                M = [mom.tile([BC, 4], f32, tag=f"M{p}", name=f"M{p}") for p in range(PT + 1)]
                N = [None] + [mom.tile([BC, 4], f32, tag=f"N{p}", name=f"N{p}") for p in range(1, PT + 1)]

                nc.vector.tensor_reduce(out=M[0][:], in_=v3, axis=AX.X, op=OP.add)
                nc.vector.tensor_reduce(out=N[1][:], in_=k3, axis=AX.X, op=OP.add)
                kv = att.tile([BC, NT], f32, tag="kv")
                nc.vector.tensor_tensor(out=kv[:], in0=k2, in1=v2, op=OP.mult)
                kv3 = kv[:].rearrange("p (h j) -> p h j", j=HS)
                nc.vector.tensor_reduce(out=M[1][:], in_=kv3, axis=AX.X, op=OP.add)
                kp = att.tile([BC, NT], f32, tag="kp")
                kp3 = kp[:].rearrange("p (h j) -> p h j", j=HS)
                nc.vector.tensor_tensor(out=kp[:], in0=k2, in1=k2, op=OP.mult)
                for p in range(2, PT + 1):
                    if p > 2:
                        nc.vector.tensor_tensor(out=kp[:], in0=kp[:], in1=k2, op=OP.mult)
                    nc.vector.tensor_reduce(out=N[p][:], in_=kp3, axis=AX.X, op=OP.add)
                    nc.vector.tensor_tensor(out=kv[:], in0=kp[:], in1=v2, op=OP.mult)
                    nc.vector.tensor_reduce(out=M[p][:], in_=kv3, axis=AX.X, op=OP.add)
                for p in range(2, PT + 1):
                    c = 1.0 / math.factorial(p)
                    nc.vector.tensor_scalar_mul(M[p][:], M[p][:], c)
                    nc.vector.tensor_scalar_mul(N[p][:], N[p][:], c)

                def bc3(m):
                    return m[:].unsqueeze(2).to_broadcast((BC, 4, HS))

                na = att.tile([BC, NT], f32, tag="na")
                na3 = na[:].rearrange("p (h j) -> p h j", j=HS)
                nc.vector.tensor_copy(na3, bc3(M[PT]))
                for p in range(PT - 1, -1, -1):
                    nc.vector.tensor_tensor(out=na[:], in0=na[:], in1=t2, op=OP.mult)
                    nc.vector.tensor_tensor(out=na3, in0=na3, in1=bc3(M[p]), op=OP.add)
                da = att.tile([BC, NT], f32, tag="da")
                da3 = da[:].rearrange("p (h j) -> p h j", j=HS)
                nc.vector.tensor_copy(da3, bc3(N[PT]))
                for p in range(PT - 1, 0, -1):
                    nc.vector.tensor_tensor(out=da[:], in0=da[:], in1=t2, op=OP.mult)
                    nc.vector.tensor_tensor(out=da3, in0=da3, in1=bc3(N[p]), op=OP.add)
                nc.vector.tensor_tensor(out=da[:], in0=da[:], in1=t2, op=OP.mult)
                nc.vector.tensor_scalar_add(da[:], da[:], float(HS))
                rec = att.tile([BC, NT], f32, tag="rec")
                nc.vector.reciprocal(rec[:], da[:])
                nc.vector.tensor_tensor(
                    out=attn[:, c0 : c0 + NT], in0=na[:], in1=rec[:], op=OP.mult
                )

                # transposes of this head group + Wo partial accumulation
                aT = []
                for j in range(4):
                    head = 4 * hg + j
                    st = aTp.tile([HS, BC], f32r, tag="aT", name="aT")
                    pst = psT.tile([HS, BC], f32, tag="tr", name="pst")
                    nc.tensor.transpose(
                        pst[:], attn[:, head * HS : (head + 1) * HS], ident[:]
                    )
                    nc.vector.tensor_copy(st[:], pst[:])
                    aT.append(st)
                for np_ in range(2):
                    for j in range(4):
                        head = 4 * hg + j
                        wt = wtp.tile([HS, 2 * NT], f32r, tag="w", name="wo_t")
                        nc.sync.dma_start(
                            wt[:],
                            wo_d[head * HS : head * HS + HS, np_ * 2 * NT : (np_ + 1) * 2 * NT],
                        )
                        nc.tensor.matmul(
                            ps_wo[2 * np_][:], aT[j][:], wt[:, 0:NT],
                            start=(hg == 0 and j == 0), stop=False,
                        )
                        nc.tensor.matmul(
                            ps_wo[2 * np_ + 1][:], aT[j][:], wt[:, NT : 2 * NT],
                            start=(hg == 0 and j == 0), stop=False,
                        )

            # ---- Wo bias row, out copies, LN2 ----
            o = acts.tile([BC, D], f32, tag="xs", name="o")  # reuse xs slot
            for np_ in range(2):
                wt = wtp.tile([1, 2 * NT], f32r, tag="w", name=f"wob{np_}")
                nc.sync.dma_start(wt[:], wo_d[D : D + 1, np_ * 2 * NT : (np_ + 1) * 2 * NT])
                nc.tensor.matmul(
                    ps_wo[2 * np_][:], ones_r[:], wt[:, 0:NT], start=False, stop=True
                )
                nc.tensor.matmul(
                    ps_wo[2 * np_ + 1][:], ones_r[:], wt[:, NT : 2 * NT],
                    start=False, stop=True,
                )
            for n in range(4):
                nc.scalar.copy(o[:, n * NT : (n + 1) * NT], ps_wo[n][:])
            h2 = acts.tile([BC, D], f32, tag="h", name="h2")  # reuse h slot
            _emit_ln(nc, lns, o[:], h2[:], D, scratch[:])
            h2T = stat_transposes(h2, "stat")

            # ---- W1 + GELU: quads of 4x392 = 1568 cols, 6272-byte lines ----
            g = acts.tile([BC, FF], f32, tag="g")
            for nq in range(4):
                q0 = nq * 4 * NT
                pss = [psB.tile([BC, NT], f32, tag="acc", name=f"psw1_{m}") for m in range(4)]
                for i, (r0, nrw, _nf) in enumerate(KT_D):
                    wt = wtp1.tile([nrw, 4 * NT], f32r, tag="w1", name="w1_t")
                    nc.sync.dma_start(wt[:], w1_d[r0 : r0 + nrw, q0 : q0 + 4 * NT])
                    for m in range(4):
                        nc.tensor.matmul(
                            pss[m][:], h2T[i][:], wt[:, m * NT : (m + 1) * NT],
                            start=i == 0, stop=i == len(KT_D) - 1,
                        )
                for m in range(4):
                    nc.scalar.activation(
                        g[:, q0 + m * NT : q0 + (m + 1) * NT], pss[m][:], AF.Gelu
                    )

            # ---- W2 (stream transposes of g), 3136-byte lines ----
            ps_w2 = [psQ.tile([BC, NT], f32, tag="mm", name=f"ps_w2{n}") for n in range(2)]
            for kk in range(NKF):
                gT = gTp.tile([128, BC], f32r, tag="gT", name="gT")
                pst = psT.tile([128, BC], f32, tag="tr", name="pst")
                nc.tensor.transpose(pst[:], g[:, kk * 128 : (kk + 1) * 128], ident[:])
                nc.vector.tensor_copy(gT[:], pst[:])
                wt = wtp.tile([128, 2 * NT], f32r, tag="w", name="w2_t")
                nc.sync.dma_start(wt[:], w2_d[kk * 128 : (kk + 1) * 128, 0 : 2 * NT])
                for n in range(2):
                    nc.tensor.matmul(
                        ps_w2[n][:], gT[:], wt[:, n * NT : (n + 1) * NT],
                        start=(kk == 0), stop=False,
                    )
            wt = wtp.tile([1, 2 * NT], f32r, tag="w", name="w2b")
            nc.sync.dma_start(wt[:], w2_d[FF : FF + 1, 0 : 2 * NT])
            for n in range(2):
                nc.tensor.matmul(
                    ps_w2[n][:], ones_r[:], wt[:, n * NT : (n + 1) * NT],
                    start=False, stop=True,
                )

            ff = acts.tile([BC, DOUT], f32, tag="ff")
            for n in range(2):
                nc.scalar.copy(ff[:, n * NT : (n + 1) * NT], ps_w2[n][:])
            nc.sync.dma_start(y_d[:], ff[:])

    nc.compile()
    return nc


def _prep_weights(Wq, Wk, Wv, Wo, bo, g1, b1, g2, b2, W1, b1f, W2, b2f):
    """Fold LN affines into adjacent weights; append bias rows. float64 exact."""
    f8 = np.float64
    wq = np.asarray(Wq, f8).transpose(1, 0, 2).reshape(D, D)
    wk = np.asarray(Wk, f8).transpose(1, 0, 2).reshape(D, D)
    wv = np.asarray(Wv, f8).transpose(1, 0, 2).reshape(D, D)
    wqkv = np.concatenate([wq, wk, wv], axis=1)  # (D, 3D)
    g1 = np.asarray(g1, f8)
    b1 = np.asarray(b1, f8)
    wqkv_aug = np.concatenate([g1[:, None] * wqkv, (b1 @ wqkv)[None, :]], axis=0)
    wo_aug = np.concatenate(
        [np.asarray(Wo, f8), np.asarray(bo, f8)[None, :]], axis=0
    )
    g2 = np.asarray(g2, f8)
    b2 = np.asarray(b2, f8)
    W1 = np.asarray(W1, f8)
    w1_aug = np.concatenate(
        [g2[:, None] * W1, (b2 @ W1 + np.asarray(b1f, f8))[None, :]], axis=0
    )
    w2_aug = np.concatenate(
        [np.asarray(W2, f8), np.asarray(b2f, f8)[None, :]], axis=0
    )
    return (
        wqkv_aug.astype(np.float32),
        wo_aug.astype(np.float32),
        w1_aug.astype(np.float32),
        w2_aug.astype(np.float32),
    )


def kernel(**inputs) -> np.ndarray:
    if "nc" not in _CACHE:
        _CACHE["nc"] = _build()
    nc = _CACHE["nc"]

    x = np.ascontiguousarray(np.asarray(inputs["x"], np.float32))
    wqkv_aug, wo_aug, w1_aug, w2_aug = _prep_weights(
        inputs["Wq"], inputs["Wk"], inputs["Wv"], inputs["Wo"], inputs["bo"],
        inputs["g1"], inputs["b1"], inputs["g2"], inputs["b2"],
        inputs["W1"], inputs["b1f"], inputs["W2"], inputs["b2f"],
    )
    in_maps = [
        {
            "x": x[c * BC : (c + 1) * BC],
            "wqkv": wqkv_aug,
            "wo": wo_aug,
            "w1": w1_aug,
            "w2": w2_aug,
        }
        for c in range(NCORES)
    ]
    res = run_bass_kernel_spmd(nc, in_maps, core_ids=list(range(NCORES)), trace=False)
    return np.concatenate([res.results[c]["y"] for c in range(NCORES)], axis=0)
